# revision 1
# baseline (speedup 1.0000x reference)
"""MultiHeadLatentAttention Trainium2 Bass kernel.

Sharding (8 cores): core c = (b, hg) with b = c // 2, hg = c % 2.
Each core handles batch b and head-group hg (8 of 16 heads):
  - QKV projection for its heads (weights pre-sliced+transposed on host)
  - qk rmsnorm + RoPE + causal attention for its 8 heads
  - pairwise AllGather of y^T (attention output) between (2b, 2b+1)
  - out-projection for c-half hg*1024:(hg+1)*1024 with the full 16 heads
Host concatenates the two c-halves per batch. No host-side math beyond
slicing/transposing weights and building constant tables.
"""

import numpy as np

import concourse.bass as bass
import concourse.mybir as mybir
import concourse.tile as tile
from concourse import bacc
from concourse.bass import ts
from concourse.masks import make_identity

F32 = mybir.dt.float32
F32R = mybir.dt.float32r
BF16 = mybir.dt.bfloat16

N_HEAD = 16
N_EMBD = 2048
N_LATENT = 1024
HEAD_DIM = 64
ROPE_BASE = 10000.0
EPS = 1e-6
N_CORES = 8

HPC = N_HEAD // 2        # heads per core = 8
DW = HPC * HEAD_DIM      # local head width = 512
TCH = 512                # t-chunk for attention moving dim


def r(ap):
    return ap


def build_nc(T=2048, C=2048, num_devices=N_CORES, debug_out=False):
    """Build the SPMD program (identical on all cores; data differs)."""
    nc = bacc.Bacc("TRN2", target_bir_lowering=False, debug=False,
                   num_devices=num_devices)

    NT = T // 128            # t-tiles
    NCT = C // 128           # c-tiles (contraction tiles for qkv proj)
    NJ = T // TCH            # t-chunks for attention
    CH = C // 2              # out c-half width = 1024
    NL = N_LATENT // 128     # l-tiles for out proj = 8
    CCW = min(512, CH)       # out column chunk
    NCC = CH // CCW

    x_d = nc.dram_tensor("x", [T, C], F32R, kind="ExternalInput").ap()
    wqT_d = nc.dram_tensor("wqT", [C, DW], F32R, kind="ExternalInput").ap()
    wkT_d = nc.dram_tensor("wkT", [C, DW], F32R, kind="ExternalInput").ap()
    wvT_d = nc.dram_tensor("wvT", [C, DW], F32R, kind="ExternalInput").ap()
    woT_d = nc.dram_tensor("woutT", [N_LATENT, CH], F32R, kind="ExternalInput").ap()
    cos_d = nc.dram_tensor("cosf", [T, DW], F32, kind="ExternalInput").ap()
    sin_d = nc.dram_tensor("sinf", [T, DW], F32, kind="ExternalInput").ap()
    mask_d = nc.dram_tensor("masks", [4, 128, TCH], F32R, kind="ExternalInput").ap()
    out_d = nc.dram_tensor("out_half", [T, CH], F32, kind="ExternalOutput").ap()
    dbg = {}
    if debug_out:
        for nm, shp in (("qtd_o", [DW, T]), ("ktd_o", [DW, T]), ("vd_o", [T, DW]),
                        ("ytl_o", [DW, T]), ("ytf_o", [2 * DW, T])):
            dbg[nm] = nc.dram_tensor(nm, shp, F32, kind="ExternalOutput").ap()

    groups = [[i, i + 1] for i in range(0, num_devices, 2)]

    with tile.TileContext(nc) as tc:
        with (
            tc.tile_pool(name="const", bufs=1) as constp,
            tc.tile_pool(name="dram", bufs=1, space=bass.MemorySpace.DRAM) as dramp,
        ):
            ident = constp.tile([128, 128], F32, tag="ident")
            make_identity(nc, ident[:])
            identr = constp.tile([128, 128], F32R, tag="identr")
            nc.vector.tensor_copy(identr[:], ident[:])
            eps_sb = constp.tile([128, 1], F32, tag="eps")
            nc.vector.memset(eps_sb[:], EPS)
            ones8 = constp.tile([128, HPC], F32, tag="ones8")
            nc.vector.memset(ones8[:], 1.0)
            ones_f = constp.tile([128, 64], F32, tag="ones_f")
            nc.vector.memset(ones_f[:], 1.0)
            onesr = constp.tile([128, 64], F32R, tag="onesr")
            nc.vector.tensor_copy(onesr[:], ones_f[:])
            mask_sb = []
            for o in range(4):
                m = constp.tile([128, TCH], F32R, tag=f"mask{o}", name=f"mask{o}")
                nc.sync.dma_start(m[:], mask_d[o])
                mask_sb.append(m)

            qtd = dramp.tile([DW, T], F32R, tag="qtd")
            ktd = dramp.tile([DW, T], F32R, tag="ktd")
            vd = dramp.tile([T, DW], F32R, tag="vd")
            ytl = dramp.tile([DW, T], F32R, tag="ytl")
            ytfs = []
            for hp in range(HPC // 2):
                yf = dramp.tile([256, T], F32R, tag=f"ytf{hp}", name=f"ytf{hp}")
                ytfs.append(yf)

            # ---------------- Phase 1: QKV + rmsnorm + rope + transpose ----
            with (
                tc.tile_pool(name="p1w", bufs=1) as p1w,
                tc.tile_pool(name="p1", bufs=2) as p1,
                tc.tile_pool(name="p1ps", bufs=2, space=bass.MemorySpace.PSUM) as p1ps,
                tc.tile_pool(name="p1qkv", bufs=2, space=bass.MemorySpace.PSUM) as p1qkv,
            ):
                wsb = {}
                for name, wd in (("q", wqT_d), ("k", wkT_d), ("v", wvT_d)):
                    w = p1w.tile([128, NCT * DW], F32R, tag=f"w{name}", name=f"w{name}")
                    nc.sync.dma_start(
                        w[:].rearrange("p (ct d) -> p ct d", d=DW),
                        wd.rearrange("(ct p) d -> p ct d", p=128),
                    )
                    wsb[name] = w

                for tt in range(NT):
                    xa = p1.tile([128, C], F32R, tag="xa")
                    nc.sync.dma_start(xa[:], x_d[ts(tt, 128), :])
                    cos_t = p1.tile([128, DW], F32, tag="cos")
                    sin_t = p1.tile([128, DW], F32, tag="sin")
                    nc.sync.dma_start(cos_t[:], cos_d[ts(tt, 128), :])
                    nc.sync.dma_start(sin_t[:], sin_d[ts(tt, 128), :])

                    # x^T for this t-tile: [c, 128t] as NCT column blocks
                    xt = p1.tile([128, NCT * 128], F32R, tag="xt")
                    for g in range((NCT + 3) // 4):
                        nblk = min(4, NCT - 4 * g)
                        xps = p1ps.tile([128, 512], F32R, tag="xps")
                        for bi in range(nblk):
                            ct = 4 * g + bi
                            nc.tensor.transpose(
                                xps[:, ts(bi, 128)], xa[:, ts(ct, 128)], identr[:]
                            )
                        nc.vector.tensor_copy(
                            xt[:, 4 * g * 128: (4 * g + nblk) * 128],
                            xps[:, : nblk * 128],
                        )

                    ps = {}
                    for name in ("q", "k", "v"):
                        p = p1qkv.tile([128, DW], F32, tag=f"ps{name}", name=f"ps{name}")
                        for ct in range(NCT):
                            nc.tensor.matmul(
                                p[:],
                                r(xt[:, ts(ct, 128)]),
                                r(wsb[name][:, ts(ct, DW)]),
                                start=(ct == 0),
                                stop=(ct == NCT - 1),
                            )
                        ps[name] = p

                    # V: evacuate to DRAM
                    vsb = p1.tile([128, DW], F32R, tag="vsb", bufs=4)
                    nc.scalar.activation(
                        vsb[:], ps["v"][:], mybir.ActivationFunctionType.Copy
                    )
                    nc.sync.dma_start(vd[ts(tt, 128), :], vsb[:])

                    # rmsnorm + rope for q, k
                    for name, dst in (("q", qtd), ("k", ktd)):
                        pq = ps[name]
                        sumsq = p1.tile([128, HPC], F32, tag="sumsq", bufs=4)
                        sqs = p1.tile([128, HEAD_DIM], F32, tag="sqs", bufs=4)
                        for h in range(HPC):
                            nc.scalar.activation(
                                sqs[:],
                                pq[:, ts(h, HEAD_DIM)],
                                mybir.ActivationFunctionType.Square,
                                accum_out=sumsq[:, h: h + 1],
                            )
                        sig = p1.tile([128, HPC], F32, tag="sig", bufs=4)
                        nc.scalar.activation(
                            sig[:], sumsq[:],
                            mybir.ActivationFunctionType.Sqrt,
                            bias=eps_sb[:], scale=1.0 / HEAD_DIM,
                        )
                        rfac = p1.tile([128, HPC], F32, tag="rfac", bufs=4)
                        nc.vector.reciprocal(rfac[:], sig[:])
                        qn = p1.tile([128, DW], F32, tag="qn", bufs=4)
                        for h in range(HPC):
                            nc.vector.tensor_scalar_mul(
                                qn[:, ts(h, HEAD_DIM)],
                                pq[:, ts(h, HEAD_DIM)],
                                rfac[:, h: h + 1],
                            )
                        # rope: qr = qn*cos + swap(qn)*sin_signed
                        qsw = p1.tile([128, DW], F32, tag="qsw", bufs=4)
                        hv = qn[:].rearrange("p (h two d) -> p h two d", two=2,
                                             d=HEAD_DIM // 2)
                        sv = qsw[:].rearrange("p (h two d) -> p h two d", two=2,
                                              d=HEAD_DIM // 2)
                        nc.vector.tensor_copy(sv[:, :, 0, :], hv[:, :, 1, :])
                        nc.vector.tensor_copy(sv[:, :, 1, :], hv[:, :, 0, :])
                        m1 = p1.tile([128, DW], F32R, tag="m1", bufs=4)
                        nc.vector.tensor_mul(m1[:], qn[:], cos_t[:])
                        m2 = p1.tile([128, DW], F32, tag="m2", bufs=4)
                        nc.vector.tensor_mul(m2[:], qsw[:], sin_t[:])
                        nc.vector.tensor_add(m1[:], m1[:], m2[:])

                        # transpose to [d, t] and store
                        qt = p1.tile([128, DW], F32R, tag="qt", bufs=4)
                        for db in range(DW // 128):
                            tps = p1ps.tile([128, 128], F32R, tag="xps")
                            nc.tensor.transpose(
                                tps[:], m1[:, ts(db, 128)], identr[:]
                            )
                            nc.vector.tensor_copy(qt[:, ts(db, 128)], tps[:])
                        for db in range(DW // 128):
                            nc.sync.dma_start(
                                dst[ts(db, 128), ts(tt, 128)], qt[:, ts(db, 128)]
                            )

            # ---------------- Phase 2: attention --------------------------
            with (
                tc.tile_pool(name="p2kv", bufs=1) as p2kv,
                tc.tile_pool(name="p2", bufs=4) as p2,
                tc.tile_pool(name="p2s", bufs=4, space=bass.MemorySpace.PSUM) as p2s,
                tc.tile_pool(name="p2y", bufs=2, space=bass.MemorySpace.PSUM) as p2y,
                tc.tile_pool(name="p2bc", bufs=2, space=bass.MemorySpace.PSUM) as p2bc,
            ):
                kts = []
                for hp in range(HPC // 2):
                    kt = p2kv.tile([128, T], F32R, tag=f"kt{hp}", name=f"kt{hp}")
                    nc.sync.dma_start(kt[:], ktd[ts(hp, 128), :])
                    kts.append(kt)
                v65 = []
                for si in range(NT):
                    v = p2kv.tile([128, HPC * 65], F32R, tag=f"v65_{si}", name=f"v65_{si}")
                    vv = v[:].rearrange("p (h e) -> p h e", e=65)
                    nc.vector.tensor_copy(
                        vv[:, :, 64:65].rearrange("p h one -> p (h one)"),
                        ones8[:])
                    nc.sync.dma_start(vv[:, :, 0:64], vd[ts(si, 128), :]
                                      .rearrange("p (h d) -> p h d", d=HEAD_DIM))
                    v65.append(v)

                for hp in range(HPC // 2):
                    for j in range(NJ):
                        q2 = p2.tile([128, TCH], F32R, tag="q2")
                        nc.sync.dma_start(q2[:], qtd[ts(hp, 128), ts(j, TCH)])
                        smax = (j + 1) * (TCH // 128)
                        pys = []
                        for e in range(2):
                            pys.append(p2y.tile([65, TCH], F32, tag="py", name=f"py{e}"))
                        for si in range(smax):
                            for e in range(2):
                                h = 2 * hp + e
                                pss = p2s.tile([128, TCH], F32, tag="pss")
                                nc.tensor.matmul(
                                    pss[:],
                                    r(kts[hp][64 * e: 64 * e + 64, ts(si, 128)]),
                                    r(q2[64 * e: 64 * e + 64, :]),
                                )
                                pt = p2.tile([128, TCH], F32R, tag="pt")
                                nc.scalar.activation(
                                    pt[:], pss[:],
                                    mybir.ActivationFunctionType.Exp,
                                    scale=1.0 / np.sqrt(HEAD_DIM),
                                )
                                o = si - (smax - TCH // 128)
                                if o >= 0:
                                    nc.gpsimd.tensor_mul(pt[:], pt[:], mask_sb[o][:])
                                nc.tensor.matmul(
                                    pys[e][:],
                                    r(v65[si][:, 65 * h: 65 * h + 65]),
                                    r(pt[:]),
                                    start=(si == 0),
                                    stop=(si == smax - 1),
                                )
                        for e in range(2):
                            h = 2 * hp + e
                            ystage = p2.tile([65, TCH], F32R, tag="ystage")
                            nc.vector.tensor_copy(ystage[:], pys[e][:])
                            bc = p2bc.tile([64, TCH], F32, tag="bc")
                            nc.tensor.matmul(
                                bc[:], onesr[64:65, :], ystage[64:65, :]
                            )
                            bcr = p2.tile([64, TCH], F32, tag="bcr")
                            nc.vector.reciprocal(bcr[:], bc[:])
                            ynt = p2.tile([64, TCH], F32R, tag="ynt")
                            nc.vector.tensor_mul(
                                ynt[:], ystage[0:64, :], bcr[:]
                            )
                            nc.sync.dma_start(
                                ytl[ts(h, HEAD_DIM), ts(j, TCH)], ynt[:]
                            )
                    nc.gpsimd.collective_compute(
                        "AllGather",
                        mybir.AluOpType.bypass,
                        replica_groups=groups,
                        ins=[ytl[ts(hp, 128), :]],
                        outs=[ytfs[hp][:]],
                    )
            if debug_out:
                nc.gpsimd.dma_start(dbg["qtd_o"], qtd[:].bitcast(F32))
                nc.gpsimd.dma_start(dbg["ktd_o"], ktd[:].bitcast(F32))
                nc.gpsimd.dma_start(dbg["vd_o"], vd[:].bitcast(F32))
                nc.gpsimd.dma_start(dbg["ytl_o"], ytl[:].bitcast(F32))
                for hp in range(HPC // 2):
                    nc.gpsimd.dma_start(
                        dbg["ytf_o"][ts(hp, 128), :],
                        ytfs[hp][0:128, :].bitcast(F32))
                    nc.gpsimd.dma_start(
                        dbg["ytf_o"][4 * 128 + hp * 128: 4 * 128 + (hp + 1) * 128, :],
                        ytfs[hp][128:256, :].bitcast(F32))

            # ---------------- Phase 3: out projection ---------------------
            with (
                tc.tile_pool(name="p3w", bufs=1) as p3w,
                tc.tile_pool(name="p3", bufs=3) as p3,
                tc.tile_pool(name="p3y", bufs=1) as p3y,
                tc.tile_pool(name="p3ps", bufs=3, space=bass.MemorySpace.PSUM) as p3ps,
            ):
                wo = p3w.tile([128, NL * CH], F32R, tag="wo")
                nc.sync.dma_start(
                    wo[:].rearrange("p (lt c) -> p lt c", c=CH),
                    woT_d.rearrange("(lt p) c -> p lt c", p=128),
                )
                yts = []
                for lt in range(NL):
                    y = p3y.tile([128, T], F32R, tag=f"yr{lt}", name=f"yr{lt}")
                    nc.sync.dma_start(
                        y[:], ytfs[lt % 4][(lt // 4) * 128:(lt // 4 + 1) * 128, :])
                    yts.append(y)
                for tt in range(NT):
                    for cc in range(NCC):
                        po = p3ps.tile([128, CCW], F32, tag="po")
                        for i, lt in enumerate(range(NL)):
                            nc.tensor.matmul(
                                po[:],
                                yts[lt][:, ts(tt, 128)],
                                wo[:, lt * CH + cc * CCW: lt * CH + (cc + 1) * CCW],
                                start=(i == 0),
                                stop=(i == NL - 1),
                            )
                        osb = p3.tile([128, CCW], F32, tag="osb")
                        nc.vector.tensor_copy(osb[:], po[:])
                        nc.sync.dma_start(
                            out_d[ts(tt, 128), ts(cc, CCW)], osb[:]
                        )

    nc.compile()
    return nc


def host_tables(T=2048):
    inv_freq = 1.0 / (ROPE_BASE ** (np.arange(0, HEAD_DIM, 2, dtype=np.float32)
                                    / HEAD_DIM))
    t = np.arange(T, dtype=np.float32)
    freqs = np.outer(t, inv_freq)
    cos = np.cos(freqs).astype(np.float32)
    sin = np.sin(freqs).astype(np.float32)
    cosf = np.tile(np.concatenate([cos, cos], axis=1), (1, HPC))
    sinf = np.tile(np.concatenate([sin, -sin], axis=1), (1, HPC))
    masks = np.zeros((4, 128, TCH), dtype=np.float32)
    for i, o in enumerate(range(0, TCH, 128)):
        masks[i] = (np.arange(TCH)[None, :] >=
                    (np.arange(128)[:, None] + o)).astype(np.float32)
    return np.ascontiguousarray(cosf), np.ascontiguousarray(sinf), masks


def make_in_maps(x, w_qkv, w_out, T=2048, num_devices=N_CORES):
    x = np.asarray(x, dtype=np.float32)
    w_qkv = np.asarray(w_qkv, dtype=np.float32)
    w_out = np.asarray(w_out, dtype=np.float32)
    C = x.shape[-1]
    cosf, sinf, masks = host_tables(T)
    in_maps = []
    for c in range(num_devices):
        b, hg = c // 2, c % 2
        sl = slice(hg * DW, (hg + 1) * DW)
        in_maps.append({
            "x": np.ascontiguousarray(x[b]),
            "wqT": np.ascontiguousarray(w_qkv[0 * N_LATENT:, :][sl].T),
            "wkT": np.ascontiguousarray(w_qkv[1 * N_LATENT:, :][sl].T),
            "wvT": np.ascontiguousarray(w_qkv[2 * N_LATENT:, :][sl].T),
            "woutT": np.ascontiguousarray(
                w_out[hg * C // 2:(hg + 1) * C // 2, :].T),
            "cosf": cosf,
            "sinf": sinf,
            "masks": masks,
        })
    return in_maps


_NC = None


def kernel(x, w_qkv, w_out):
    global _NC
    if _NC is None:
        _NC = build_nc()
    from concourse.bass_utils import run_bass_kernel_spmd
    in_maps = make_in_maps(x, w_qkv, w_out)
    res = run_bass_kernel_spmd(_NC, in_maps, list(range(N_CORES))).results
    B, T = 4, 2048
    out = np.empty((B, T, N_EMBD), dtype=np.float32)
    for c in range(N_CORES):
        b, hg = c // 2, c % 2
        out[b, :, hg * N_EMBD // 2:(hg + 1) * N_EMBD // 2] = res[c]["out_half"]
    return out

